# revision 17
# baseline (speedup 1.0000x reference)
"""Trainium2 Bass kernel for DescriptorMatcherWithSteerer (nn matching).

For each row of desc1, find the nearest neighbor in desc2 under the
minimum-over-steered-copies L2 distance:

    dm[i,j]  = min_s sqrt(max(0, q2_s[i] + b2[j] - 2 * a_s[i]·b[j]))
    a_s      = desc1 @ (G.T)^s,   s = 0..order-1

Sharding: desc1 rows split across 8 NeuronCores (data parallel over queries);
desc2 (transposed) and norms replicated. Host precomputes steered descriptors
(transposed, scaled by -2) and norms.

Device strategy (per core): fp32r (replicated-mode, ~tf32-accuracy) matmuls on
PE produce -2*a_s·b in PSUM; ACT forms psum+q2 for odd steers, DVE
scalar_tensor_tensor fuses form+min for even steers, GPSIMD adds b2, DVE
reduces each 1024-column group to a per-group min. The device returns ONLY the
per-(row, group) min strip; the host finds each row's competitive groups
(within an error margin) and re-ranks them exactly in fp32, yielding exact
argmin indices and fp32-exact distances.
"""

import numpy as np

_PATCHED = [False]
_CACHE = {}

D = 128          # descriptor dim == PE contraction dim
TILE_P = 128     # query rows per tile (partition dim)
GB = 1024        # desc2 columns per group (PSUM tile width; 2 matmuls)
MMW = 512        # matmul output cols per PSUM bank (fp32 psum)
EPS = 2.5        # host rescue margin (fp32r matmul + fp16 pipeline rounding)


def _apply_patches():
    """Walrus in this container accepts only ONE sync-wait command per
    instruction; TileContext's tail drain carries one wait per outstanding
    semaphore. Spread the tail waits across a NOP chain."""
    if _PATCHED[0]:
        return
    import concourse.mybir as mybir
    import concourse.tile as tile
    from concourse.vector_clock import ScopedClock

    def _patched_drain_and_barrier(self, tick_clock, wait_clock):
        nc = self.nc
        wait_nop = nc.sync.nop(nofuse=True, hint="tile_tail_waits")
        wait_clock.add_sem_waits(
            wait_nop.ins, ScopedClock({None: tick_clock.global_clock})
        )
        si = wait_nop.ins.sync_info
        if si is not None and si.on_wait and len(si.on_wait) > 1:
            waits = list(si.on_wait)
            si.on_wait = waits[:1]
            for w in waits[1:]:
                extra = nc.sync.nop(nofuse=True, hint="tile_tail_waits")
                xsi = extra.ins.sync_info
                if xsi is None:
                    extra.ins.sync_info = mybir.SyncInfo(on_wait=[w], on_update=[])
                else:
                    xsi.on_wait = [w]
        nc.sync.drain()
        nc.all_engine_barrier()
        assert self.sems is not None
        popped = nc._tile_sem_poison_stack.pop()
        assert popped is self._sem_poison
        nc.clear_and_free_semaphores(list(self.sems.allocated().values()))
        nc.all_engine_barrier()

    tile.TileContext._drain_and_barrier = _patched_drain_and_barrier
    _PATCHED[0] = True


def _split_multi_waits(nc):
    """Move extra sync-waits onto same-engine NoOps placed right before the
    carrying instruction (same per-engine program order => same gating)."""
    import concourse.mybir as mybir

    n = [0]
    for f in nc.m.functions:
        for bb in f.blocks:
            out = []
            changed = False
            for inst in bb.instructions:
                si = inst.sync_info
                if si is not None and si.on_wait and len(si.on_wait) > 1:
                    waits = list(si.on_wait)
                    si.on_wait = waits[:1]
                    for w in waits[1:]:
                        n[0] += 1
                        nop = mybir.InstNoOp(
                            name=f"waitsplit-{n[0]}",
                            engine=inst.engine,
                            ins=[],
                            outs=[],
                        )
                        nop.sync_info = mybir.SyncInfo(on_wait=[w], on_update=[])
                        out.append(nop)
                    changed = True
                out.append(inst)
            if changed:
                bb.instructions = out


# ----------------------------------------------------------------------------
# Device program
# ----------------------------------------------------------------------------


def build_program(order: int, rows: int, b2n: int, split_waits: bool = True):
    """Per-core program.

    Inputs (per core):
      a4t  (order, 128, rows) bf16 : steered query slabs, transposed, scaled -2
      d2t  (128, b2n)         bf16 : desc2 transposed
      b2r  (128, b2n)         f16  : desc2 squared norms, replicated over rows
      q2p  (128, order*NT)    f32  : query norms, tiled layout [i, s*NT+t]
    Output:
      cmin (NT*128, NG) f16 : per-(row, group) min of d^2 over all steers.
    """
    _apply_patches()
    import concourse.bass as bass
    import concourse.mybir as mybir
    import concourse.tile as tile

    f32 = mybir.dt.float32
    bf16 = mybir.dt.bfloat16
    f16 = mybir.dt.float16
    Alu = mybir.AluOpType
    Act = mybir.ActivationFunctionType

    NT = (rows + TILE_P - 1) // TILE_P
    NG = (b2n + GB - 1) // GB

    nc = bass.Bass("TRN2", target_bir_lowering=False, debug=False)
    a4t = nc.dram_tensor("a4t", [order, D, rows], bf16, kind="ExternalInput").ap()
    d2t = nc.dram_tensor("d2t", [D, b2n], bf16, kind="ExternalInput").ap()
    b2r = nc.dram_tensor("b2r", [TILE_P, b2n], f16, kind="ExternalInput").ap()
    q2p = nc.dram_tensor("q2p", [TILE_P, order * NT], f32, kind="ExternalInput").ap()
    cmin_d = nc.dram_tensor(
        "cmin", [NT * TILE_P, NG], f16, kind="ExternalOutput"
    ).ap()

    with tile.TileContext(nc) as tc:
        with (
            tc.tile_pool(name="const", bufs=1) as cpool,
            tc.tile_pool(name="work", bufs=3) as wpool,
            tc.tile_pool(name="psum", bufs=2, space="PSUM") as ppool,
        ):
            d2t_sb = cpool.tile([D, b2n], bf16, tag="d2t")
            nc.sync.dma_start(d2t_sb[:], d2t[:, :])
            b2r_sb = cpool.tile([TILE_P, b2n], f16, tag="b2r")
            nc.sync.dma_start(b2r_sb[:], b2r[:, :])
            q2_sb = cpool.tile([TILE_P, order * NT], f32, tag="q2p")
            nc.sync.dma_start(q2_sb[:], q2p[:, :])
            a_sb = []
            for s in range(order):
                t_ = cpool.tile([D, rows], bf16, tag=f"a{s}")
                nc.sync.dma_start(t_[:], a4t[s])
                a_sb.append(t_)

            for t in range(NT):
                P = min(TILE_P, rows - t * TILE_P)
                i0 = t * TILE_P
                strip = wpool.tile([TILE_P, NG], f16, tag="strip")

                def q2ap(s, P=P, t=t):
                    return q2_sb[:P, s * NT + t : s * NT + t + 1]

                def mm_group(p_, s, g0, w, P=P, i0=i0):
                    # fill (128, w) psum tile with -2*a_s . b[g0:g0+w]
                    for off in range(0, w, MMW):
                        ww = min(MMW, w - off)
                        nc.tensor.matmul(
                            p_[:P, off : off + ww],
                            a_sb[s][:, i0 : i0 + P],
                            d2t_sb[:, g0 + off : g0 + off + ww],
                            start=True,
                            stop=True,
                        )

                for g in range(NG):
                    g0 = g * GB
                    w = min(GB, b2n - g0)
                    if order == 4:
                        # half A (steers 0,1): ACT forms s1; DVE STT fuses
                        # s0's form with the pair min -> mA (f16)
                        pa = ppool.tile([TILE_P, GB], f32, tag="pgA")
                        mm_group(pa, 0, g0, w)
                        pb = ppool.tile([TILE_P, GB], f32, tag="pgB")
                        mm_group(pb, 1, g0, w)
                        t1 = wpool.tile([TILE_P, GB], f16, tag="t1")
                        nc.scalar.activation(
                            t1[:P, :w], pb[:P, :w], Act.Identity, bias=q2ap(1)
                        )
                        mA = wpool.tile([TILE_P, GB], f16, tag="mA")
                        if g % 3 == 2:
                            # ACT-heavy variant: form s0 on ACT, min on DVE f16
                            t0 = wpool.tile([TILE_P, GB], f16, tag="t0")
                            nc.scalar.activation(
                                t0[:P, :w], pa[:P, :w], Act.Identity, bias=q2ap(0)
                            )
                            nc.vector.tensor_tensor(
                                mA[:P, :w], t0[:P, :w], t1[:P, :w], Alu.min
                            )
                        else:
                            nc.vector.scalar_tensor_tensor(
                                mA[:P, :w], pa[:P, :w], q2ap(0), t1[:P, :w],
                                Alu.add, Alu.min,
                            )
                        # half B (steers 2,3): ACT forms both; DVE f16 2x min
                        pa2 = ppool.tile([TILE_P, GB], f32, tag="pgA")
                        mm_group(pa2, 2, g0, w)
                        pb2 = ppool.tile([TILE_P, GB], f32, tag="pgB")
                        mm_group(pb2, 3, g0, w)
                        t2 = wpool.tile([TILE_P, GB], f16, tag="t2")
                        nc.scalar.activation(
                            t2[:P, :w], pa2[:P, :w], Act.Identity, bias=q2ap(2)
                        )
                        t3 = wpool.tile([TILE_P, GB], f16, tag="t3")
                        nc.scalar.activation(
                            t3[:P, :w], pb2[:P, :w], Act.Identity, bias=q2ap(3)
                        )
                        mB = wpool.tile([TILE_P, GB], f16, tag="mB")
                        nc.vector.tensor_tensor(
                            mB[:P, :w], t2[:P, :w], t3[:P, :w], Alu.min
                        )
                        u = wpool.tile([TILE_P, GB], f16, tag="u")
                        nc.vector.tensor_tensor(
                            u[:P, :w], mA[:P, :w], mB[:P, :w], Alu.min
                        )
                        d2c = wpool.tile([TILE_P, GB], f16, tag="d2c")
                        nc.gpsimd.tensor_tensor(
                            d2c[:P, :w], u[:P, :w], b2r_sb[:P, g0 : g0 + w], Alu.add
                        )
                        nc.vector.tensor_reduce(
                            strip[:P, g : g + 1], d2c[:P, :w],
                            mybir.AxisListType.X, Alu.min,
                        )
                    else:
                        # generic: single DVE min-chain over all steers (f32)
                        pa = ppool.tile([TILE_P, GB], f32, tag="pgA")
                        mm_group(pa, 0, g0, w)
                        uu = wpool.tile([TILE_P, GB], f32, tag="mA")
                        nc.vector.tensor_scalar(
                            uu[:P, :w], pa[:P, :w], q2ap(0), None, Alu.add
                        )
                        for s in range(1, order):
                            pb = ppool.tile([TILE_P, GB], f32, tag="pgB")
                            mm_group(pb, s, g0, w)
                            nc.vector.scalar_tensor_tensor(
                                uu[:P, :w], pb[:P, :w], q2ap(s), uu[:P, :w],
                                Alu.add, Alu.min,
                            )
                        d2c = wpool.tile([TILE_P, GB], f16, tag="d2c")
                        nc.vector.tensor_tensor(
                            d2c[:P, :w], uu[:P, :w], b2r_sb[:P, g0 : g0 + w], Alu.add
                        )
                        nc.vector.tensor_reduce(
                            strip[:P, g : g + 1], d2c[:P, :w],
                            mybir.AxisListType.X, Alu.min,
                        )

                nc.sync.dma_start(cmin_d[i0 : i0 + P, :], strip[:P, :])

    if split_waits:
        _split_multi_waits(nc)
    return nc


# ----------------------------------------------------------------------------
# Host side
# ----------------------------------------------------------------------------


def _host_prep(desc1, desc2, generator, order, n_cores):
    B1 = desc1.shape[0]
    B2 = desc2.shape[0]
    rows = B1 // n_cores
    NT = (rows + TILE_P - 1) // TILE_P

    a = desc1.astype(np.float32, copy=False)
    gT = generator.T.astype(np.float32, copy=False)
    steered = []
    for s in range(order):
        steered.append(a)
        if s + 1 < order:
            a = a @ gT
    q2 = np.stack([(x.astype(np.float32) ** 2).sum(1) for x in steered])  # (S, B1)
    b2 = (desc2.astype(np.float32) ** 2).sum(1)  # (B2,)

    import ml_dtypes

    d2t = np.ascontiguousarray(desc2.T.astype(ml_dtypes.bfloat16))
    b2r = np.ascontiguousarray(np.broadcast_to(b2, (TILE_P, B2)), dtype=np.float16)

    in_maps = []
    for k in range(n_cores):
        sl = slice(k * rows, (k + 1) * rows)
        a4t = np.stack(
            [np.ascontiguousarray((-2.0 * x[sl]).T.astype(ml_dtypes.bfloat16))
             for x in steered]
        )
        q2k = q2[:, sl]  # (S, rows)
        pad = NT * TILE_P - rows
        if pad:
            q2k = np.pad(q2k, ((0, 0), (0, pad)))
        q2p = np.ascontiguousarray(
            q2k.reshape(order, NT, TILE_P).transpose(2, 0, 1).reshape(TILE_P, order * NT)
        )
        in_maps.append({"a4t": a4t, "d2t": d2t, "b2r": b2r, "q2p": q2p})

    prep = dict(steered=steered, q2=q2, b2=b2, desc2=desc2, rows=rows, NT=NT,
                order=order, B1=B1, B2=B2, n_cores=n_cores)
    return in_maps, prep


def _postprocess(cmin_list, prep):
    """Exact re-rank: per row find groups whose approx min is within EPS of
    the row's global approx min, recompute those groups' d^2 exactly (fp32),
    take exact min + first argmin."""
    B1, B2 = prep["B1"], prep["B2"]
    rows, order = prep["rows"], prep["order"]
    steered, q2, b2, desc2 = prep["steered"], prep["q2"], prep["b2"], prep["desc2"]
    NG = (B2 + GB - 1) // GB

    strip = np.empty((B1, NG), np.float32)
    for k, cm in enumerate(cmin_list):
        strip[k * rows : (k + 1) * rows] = cm[:rows].astype(np.float32)

    gmin = strip.min(1)
    mask = strip <= (gmin + EPS)[:, None]

    best_v = np.full(B1, np.inf, np.float32)
    best_j = np.zeros(B1, np.int64)
    d2tT = desc2.T.astype(np.float32)  # (128, B2)
    for g in range(NG):
        ridx = np.nonzero(mask[:, g])[0]
        if ridx.size == 0:
            continue
        g0 = g * GB
        w = min(GB, B2 - g0)
        bT = d2tT[:, g0 : g0 + w]  # (128, w)
        blk = None
        for s in range(order):
            sv = steered[s][ridx]  # (n, 128)
            d2 = q2[s][ridx][:, None] + b2[None, g0 : g0 + w] - 2.0 * (sv @ bT)
            blk = d2 if blk is None else np.minimum(blk, d2)
        v = blk.min(1)
        j = blk.argmin(1) + g0
        upd = v < best_v[ridx]
        bi = ridx[upd]
        best_v[bi] = v[upd]
        best_j[bi] = j[upd]

    dists = np.sqrt(np.clip(best_v, 0.0, None)).astype(np.float32)
    return dists, best_j.astype(np.int32)


def run_cores(desc1, desc2, generator, order, n_cores=8, trace=False, trace_kwargs=None):
    from concourse.bass_utils import run_bass_kernel_spmd

    B2 = desc2.shape[0]
    in_maps, prep = _host_prep(desc1, desc2, generator, order, n_cores)

    key = (order, prep["rows"], B2)
    if key not in _CACHE:
        _CACHE[key] = build_program(order, prep["rows"], B2)
    nc = _CACHE[key]

    kw = {}
    if trace:
        kw = dict(trace=True, trace_kwargs=trace_kwargs or {})
    res = run_bass_kernel_spmd(nc, in_maps, core_ids=list(range(n_cores)), **kw)

    cmin_list = [res.results[k]["cmin"] for k in range(n_cores)]
    dists, idx, = _postprocess(cmin_list, prep)
    return dists, idx, res


def kernel(desc1, desc2, generator, steerer_order):
    order = int(steerer_order)
    desc1 = np.asarray(desc1, dtype=np.float32)
    desc2 = np.asarray(desc2, dtype=np.float32)
    generator = np.asarray(generator, dtype=np.float32)

    dists, idx, _ = run_cores(desc1, desc2, generator, order, n_cores=8)

    B1 = desc1.shape[0]
    idxs_in_1 = np.arange(B1, dtype=np.int32)
    matches = np.stack([idxs_in_1, idx], axis=1)
    return dists[:, None], matches


# revision 19
# speedup vs baseline: 1.0852x; 1.0852x over previous
"""Trainium2 Bass kernel for DescriptorMatcherWithSteerer (nn matching).

For each row of desc1, find the nearest neighbor in desc2 under the
minimum-over-steered-copies L2 distance:

    dm[i,j]  = min_s sqrt(max(0, q2_s[i] + b2[j] - 2 * a_s[i]·b[j]))
    a_s      = desc1 @ (G.T)^s,   s = 0..order-1

Sharding: desc1 rows split across 8 NeuronCores (data parallel over queries);
desc2 (transposed) and norms replicated. Host precomputes steered descriptors
(transposed, scaled by -2) and norms.

Device strategy (per core): fp32r (replicated-mode, ~tf32-accuracy) matmuls on
PE produce -2*a_s·b in PSUM; ACT forms psum+q2 for odd steers, DVE
scalar_tensor_tensor fuses form+min for even steers, GPSIMD adds b2, DVE
reduces each 1024-column group to a per-group min. The device returns ONLY the
per-(row, group) min strip; the host finds each row's competitive groups
(within an error margin) and re-ranks them exactly in fp32, yielding exact
argmin indices and fp32-exact distances.
"""

import numpy as np

_PATCHED = [False]
_CACHE = {}

D = 128          # descriptor dim == PE contraction dim
TILE_P = 128     # query rows per tile (partition dim)
GB = 1024        # desc2 columns per group (PSUM tile width; 2 matmuls)
MMW = 512        # matmul output cols per PSUM bank (fp32 psum)
EPS = 2.5        # host rescue margin (fp32r matmul + fp16 pipeline rounding)


def _apply_patches():
    """Walrus in this container accepts only ONE sync-wait command per
    instruction; TileContext's tail drain carries one wait per outstanding
    semaphore. Spread the tail waits across a NOP chain."""
    if _PATCHED[0]:
        return
    import concourse.mybir as mybir
    import concourse.tile as tile
    from concourse.vector_clock import ScopedClock

    def _patched_drain_and_barrier(self, tick_clock, wait_clock):
        nc = self.nc
        wait_nop = nc.sync.nop(nofuse=True, hint="tile_tail_waits")
        wait_clock.add_sem_waits(
            wait_nop.ins, ScopedClock({None: tick_clock.global_clock})
        )
        si = wait_nop.ins.sync_info
        if si is not None and si.on_wait and len(si.on_wait) > 1:
            waits = list(si.on_wait)
            si.on_wait = waits[:1]
            for w in waits[1:]:
                extra = nc.sync.nop(nofuse=True, hint="tile_tail_waits")
                xsi = extra.ins.sync_info
                if xsi is None:
                    extra.ins.sync_info = mybir.SyncInfo(on_wait=[w], on_update=[])
                else:
                    xsi.on_wait = [w]
        nc.sync.drain()
        nc.all_engine_barrier()
        assert self.sems is not None
        popped = nc._tile_sem_poison_stack.pop()
        assert popped is self._sem_poison
        nc.clear_and_free_semaphores(list(self.sems.allocated().values()))
        nc.all_engine_barrier()

    tile.TileContext._drain_and_barrier = _patched_drain_and_barrier
    _PATCHED[0] = True


def _split_multi_waits(nc):
    """Move extra sync-waits onto same-engine NoOps placed right before the
    carrying instruction (same per-engine program order => same gating)."""
    import concourse.mybir as mybir

    n = [0]
    for f in nc.m.functions:
        for bb in f.blocks:
            out = []
            changed = False
            for inst in bb.instructions:
                si = inst.sync_info
                if si is not None and si.on_wait and len(si.on_wait) > 1:
                    waits = list(si.on_wait)
                    si.on_wait = waits[:1]
                    for w in waits[1:]:
                        n[0] += 1
                        nop = mybir.InstNoOp(
                            name=f"waitsplit-{n[0]}",
                            engine=inst.engine,
                            ins=[],
                            outs=[],
                        )
                        nop.sync_info = mybir.SyncInfo(on_wait=[w], on_update=[])
                        out.append(nop)
                    changed = True
                out.append(inst)
            if changed:
                bb.instructions = out


# ----------------------------------------------------------------------------
# Device program
# ----------------------------------------------------------------------------


def build_program(order: int, rows: int, b2n: int, split_waits: bool = True):
    """Per-core program.

    Inputs (per core):
      a4t  (order, 128, rows) bf16 : steered query slabs, transposed, scaled -2
      d2t  (128, b2n)         bf16 : desc2 transposed
      b2r  (128, b2n)         f16  : desc2 squared norms, replicated over rows
      q2p  (128, order*NT)    f32  : query norms, tiled layout [i, s*NT+t]
    Output:
      cmin (NT*128, NG) f16 : per-(row, group) min of d^2 over all steers.
    """
    _apply_patches()
    import concourse.bass as bass
    import concourse.mybir as mybir
    import concourse.tile as tile

    f32 = mybir.dt.float32
    bf16 = mybir.dt.bfloat16
    f16 = mybir.dt.float16
    Alu = mybir.AluOpType
    Act = mybir.ActivationFunctionType

    NT = (rows + TILE_P - 1) // TILE_P
    NG = (b2n + GB - 1) // GB

    nc = bass.Bass("TRN2", target_bir_lowering=False, debug=False)
    a4t = nc.dram_tensor("a4t", [order, D, rows], bf16, kind="ExternalInput").ap()
    d2t = nc.dram_tensor("d2t", [D, b2n], bf16, kind="ExternalInput").ap()
    b2r = nc.dram_tensor("b2r", [TILE_P, b2n], f16, kind="ExternalInput").ap()
    q2p = nc.dram_tensor("q2p", [TILE_P, order * NT], f32, kind="ExternalInput").ap()
    cmin_d = nc.dram_tensor(
        "cmin", [NT * TILE_P, NG], f16, kind="ExternalOutput"
    ).ap()

    with tile.TileContext(nc) as tc:
        with (
            tc.tile_pool(name="const", bufs=1) as cpool,
            tc.tile_pool(name="work", bufs=4) as wpool,
            tc.tile_pool(name="psum", bufs=2, space="PSUM") as ppool,
        ):
            d2t_sb = cpool.tile([D, b2n], bf16, tag="d2t")
            nc.sync.dma_start(d2t_sb[:], d2t[:, :])
            b2r_sb = cpool.tile([TILE_P, b2n], f16, tag="b2r")
            nc.sync.dma_start(b2r_sb[:], b2r[:, :])
            q2_sb = cpool.tile([TILE_P, order * NT], f32, tag="q2p")
            nc.sync.dma_start(q2_sb[:], q2p[:, :])
            a_sb = []
            for s in range(order):
                t_ = cpool.tile([D, rows], bf16, tag=f"a{s}")
                nc.sync.dma_start(t_[:], a4t[s])
                a_sb.append(t_)

            for t in range(NT):
                P = min(TILE_P, rows - t * TILE_P)
                i0 = t * TILE_P
                strip = wpool.tile([TILE_P, NG], f16, tag="strip")

                def q2ap(s, P=P, t=t):
                    return q2_sb[:P, s * NT + t : s * NT + t + 1]

                def mm_group(p_, s, g0, w, P=P, i0=i0):
                    # fill (128, w) psum tile with -2*a_s . b[g0:g0+w]
                    for off in range(0, w, MMW):
                        ww = min(MMW, w - off)
                        nc.tensor.matmul(
                            p_[:P, off : off + ww],
                            a_sb[s][:, i0 : i0 + P],
                            d2t_sb[:, g0 + off : g0 + off + ww],
                            start=True,
                            stop=True,
                        )

                for g in range(NG):
                    g0 = g * GB
                    w = min(GB, b2n - g0)
                    if order == 4:
                        # half A (steers 0,1): ACT forms s1; DVE STT fuses
                        # s0's form with the pair min -> mA (f16)
                        pa = ppool.tile([TILE_P, GB], f32, tag="pgA")
                        mm_group(pa, 0, g0, w)
                        pb = ppool.tile([TILE_P, GB], f32, tag="pgB")
                        mm_group(pb, 1, g0, w)
                        t1 = wpool.tile([TILE_P, GB], f16, tag="t1")
                        nc.scalar.activation(
                            t1[:P, :w], pb[:P, :w], Act.Identity, bias=q2ap(1)
                        )
                        mA = wpool.tile([TILE_P, GB], f16, tag="mA")
                        nc.vector.scalar_tensor_tensor(
                            mA[:P, :w], pa[:P, :w], q2ap(0), t1[:P, :w],
                            Alu.add, Alu.min,
                        )
                        # half B (steers 2,3): ACT forms both; DVE f16 2x min
                        pa2 = ppool.tile([TILE_P, GB], f32, tag="pgA")
                        mm_group(pa2, 2, g0, w)
                        pb2 = ppool.tile([TILE_P, GB], f32, tag="pgB")
                        mm_group(pb2, 3, g0, w)
                        t2 = wpool.tile([TILE_P, GB], f16, tag="t2")
                        nc.scalar.activation(
                            t2[:P, :w], pa2[:P, :w], Act.Identity, bias=q2ap(2)
                        )
                        t3 = wpool.tile([TILE_P, GB], f16, tag="t3")
                        nc.scalar.activation(
                            t3[:P, :w], pb2[:P, :w], Act.Identity, bias=q2ap(3)
                        )
                        mB = wpool.tile([TILE_P, GB], f16, tag="mB")
                        nc.vector.tensor_tensor(
                            mB[:P, :w], t2[:P, :w], t3[:P, :w], Alu.min
                        )
                        u = wpool.tile([TILE_P, GB], f16, tag="u")
                        nc.vector.tensor_tensor(
                            u[:P, :w], mA[:P, :w], mB[:P, :w], Alu.min
                        )
                        d2c = wpool.tile([TILE_P, GB], f16, tag="d2c")
                        nc.gpsimd.tensor_tensor(
                            d2c[:P, :w], u[:P, :w], b2r_sb[:P, g0 : g0 + w], Alu.add
                        )
                        nc.vector.tensor_reduce(
                            strip[:P, g : g + 1], d2c[:P, :w],
                            mybir.AxisListType.X, Alu.min,
                        )
                    else:
                        # generic: single DVE min-chain over all steers (f32)
                        pa = ppool.tile([TILE_P, GB], f32, tag="pgA")
                        mm_group(pa, 0, g0, w)
                        uu = wpool.tile([TILE_P, GB], f32, tag="mA")
                        nc.vector.tensor_scalar(
                            uu[:P, :w], pa[:P, :w], q2ap(0), None, Alu.add
                        )
                        for s in range(1, order):
                            pb = ppool.tile([TILE_P, GB], f32, tag="pgB")
                            mm_group(pb, s, g0, w)
                            nc.vector.scalar_tensor_tensor(
                                uu[:P, :w], pb[:P, :w], q2ap(s), uu[:P, :w],
                                Alu.add, Alu.min,
                            )
                        d2c = wpool.tile([TILE_P, GB], f16, tag="d2c")
                        nc.vector.tensor_tensor(
                            d2c[:P, :w], uu[:P, :w], b2r_sb[:P, g0 : g0 + w], Alu.add
                        )
                        nc.vector.tensor_reduce(
                            strip[:P, g : g + 1], d2c[:P, :w],
                            mybir.AxisListType.X, Alu.min,
                        )

                nc.sync.dma_start(cmin_d[i0 : i0 + P, :], strip[:P, :])

    if split_waits:
        _split_multi_waits(nc)
    return nc


# ----------------------------------------------------------------------------
# Host side
# ----------------------------------------------------------------------------


def _host_prep(desc1, desc2, generator, order, n_cores):
    B1 = desc1.shape[0]
    B2 = desc2.shape[0]
    rows = B1 // n_cores
    NT = (rows + TILE_P - 1) // TILE_P

    a = desc1.astype(np.float32, copy=False)
    gT = generator.T.astype(np.float32, copy=False)
    steered = []
    for s in range(order):
        steered.append(a)
        if s + 1 < order:
            a = a @ gT
    q2 = np.stack([(x.astype(np.float32) ** 2).sum(1) for x in steered])  # (S, B1)
    b2 = (desc2.astype(np.float32) ** 2).sum(1)  # (B2,)

    import ml_dtypes

    d2t = np.ascontiguousarray(desc2.T.astype(ml_dtypes.bfloat16))
    b2r = np.ascontiguousarray(np.broadcast_to(b2, (TILE_P, B2)), dtype=np.float16)

    in_maps = []
    for k in range(n_cores):
        sl = slice(k * rows, (k + 1) * rows)
        a4t = np.stack(
            [np.ascontiguousarray((-2.0 * x[sl]).T.astype(ml_dtypes.bfloat16))
             for x in steered]
        )
        q2k = q2[:, sl]  # (S, rows)
        pad = NT * TILE_P - rows
        if pad:
            q2k = np.pad(q2k, ((0, 0), (0, pad)))
        q2p = np.ascontiguousarray(
            q2k.reshape(order, NT, TILE_P).transpose(2, 0, 1).reshape(TILE_P, order * NT)
        )
        in_maps.append({"a4t": a4t, "d2t": d2t, "b2r": b2r, "q2p": q2p})

    prep = dict(steered=steered, q2=q2, b2=b2, desc2=desc2, rows=rows, NT=NT,
                order=order, B1=B1, B2=B2, n_cores=n_cores)
    return in_maps, prep


def _postprocess(cmin_list, prep):
    """Exact re-rank: per row find groups whose approx min is within EPS of
    the row's global approx min, recompute those groups' d^2 exactly (fp32),
    take exact min + first argmin."""
    B1, B2 = prep["B1"], prep["B2"]
    rows, order = prep["rows"], prep["order"]
    steered, q2, b2, desc2 = prep["steered"], prep["q2"], prep["b2"], prep["desc2"]
    NG = (B2 + GB - 1) // GB

    strip = np.empty((B1, NG), np.float32)
    for k, cm in enumerate(cmin_list):
        strip[k * rows : (k + 1) * rows] = cm[:rows].astype(np.float32)

    gmin = strip.min(1)
    mask = strip <= (gmin + EPS)[:, None]

    best_v = np.full(B1, np.inf, np.float32)
    best_j = np.zeros(B1, np.int64)
    d2tT = desc2.T.astype(np.float32)  # (128, B2)
    for g in range(NG):
        ridx = np.nonzero(mask[:, g])[0]
        if ridx.size == 0:
            continue
        g0 = g * GB
        w = min(GB, B2 - g0)
        bT = d2tT[:, g0 : g0 + w]  # (128, w)
        blk = None
        for s in range(order):
            sv = steered[s][ridx]  # (n, 128)
            d2 = q2[s][ridx][:, None] + b2[None, g0 : g0 + w] - 2.0 * (sv @ bT)
            blk = d2 if blk is None else np.minimum(blk, d2)
        v = blk.min(1)
        j = blk.argmin(1) + g0
        upd = v < best_v[ridx]
        bi = ridx[upd]
        best_v[bi] = v[upd]
        best_j[bi] = j[upd]

    dists = np.sqrt(np.clip(best_v, 0.0, None)).astype(np.float32)
    return dists, best_j.astype(np.int32)


def run_cores(desc1, desc2, generator, order, n_cores=8, trace=False, trace_kwargs=None):
    from concourse.bass_utils import run_bass_kernel_spmd

    B2 = desc2.shape[0]
    in_maps, prep = _host_prep(desc1, desc2, generator, order, n_cores)

    key = (order, prep["rows"], B2)
    if key not in _CACHE:
        _CACHE[key] = build_program(order, prep["rows"], B2)
    nc = _CACHE[key]

    kw = {}
    if trace:
        kw = dict(trace=True, trace_kwargs=trace_kwargs or {})
    res = run_bass_kernel_spmd(nc, in_maps, core_ids=list(range(n_cores)), **kw)

    cmin_list = [res.results[k]["cmin"] for k in range(n_cores)]
    dists, idx, = _postprocess(cmin_list, prep)
    return dists, idx, res


def kernel(desc1, desc2, generator, steerer_order):
    order = int(steerer_order)
    desc1 = np.asarray(desc1, dtype=np.float32)
    desc2 = np.asarray(desc2, dtype=np.float32)
    generator = np.asarray(generator, dtype=np.float32)

    dists, idx, _ = run_cores(desc1, desc2, generator, order, n_cores=8)

    B1 = desc1.shape[0]
    idxs_in_1 = np.arange(B1, dtype=np.int32)
    matches = np.stack([idxs_in_1, idx], axis=1)
    return dists[:, None], matches


# revision 20
# speedup vs baseline: 1.0915x; 1.0058x over previous
"""Trainium2 Bass kernel for DescriptorMatcherWithSteerer (nn matching).

For each row of desc1, find the nearest neighbor in desc2 under the
minimum-over-steered-copies L2 distance:

    dm[i,j]  = min_s sqrt(max(0, q2_s[i] + b2[j] - 2 * a_s[i]·b[j]))
    a_s      = desc1 @ (G.T)^s,   s = 0..order-1

Sharding: desc1 rows split across 8 NeuronCores (data parallel over queries);
desc2 (transposed) and norms replicated. Host precomputes steered descriptors
(transposed, scaled by -2) and norms.

Device strategy (per core): fp32r (replicated-mode, ~tf32-accuracy) matmuls on
PE produce -2*a_s·b in PSUM; ACT forms psum+q2 for odd steers, DVE
scalar_tensor_tensor fuses form+min for even steers, GPSIMD adds b2, DVE
reduces each 1024-column group to a per-group min. The device returns ONLY the
per-(row, group) min strip; the host finds each row's competitive groups
(within an error margin) and re-ranks them exactly in fp32, yielding exact
argmin indices and fp32-exact distances.
"""

import numpy as np

_PATCHED = [False]
_CACHE = {}

D = 128          # descriptor dim == PE contraction dim
TILE_P = 128     # query rows per tile (partition dim)
GB = 1024        # desc2 columns per group (PSUM tile width; 2 matmuls)
MMW = 512        # matmul output cols per PSUM bank (fp32 psum)
EPS = 2.5        # host rescue margin (fp32r matmul + fp16 pipeline rounding)


def _apply_patches():
    """Walrus in this container accepts only ONE sync-wait command per
    instruction; TileContext's tail drain carries one wait per outstanding
    semaphore. Spread the tail waits across a NOP chain."""
    if _PATCHED[0]:
        return
    import concourse.mybir as mybir
    import concourse.tile as tile
    from concourse.vector_clock import ScopedClock

    def _patched_drain_and_barrier(self, tick_clock, wait_clock):
        nc = self.nc
        wait_nop = nc.sync.nop(nofuse=True, hint="tile_tail_waits")
        wait_clock.add_sem_waits(
            wait_nop.ins, ScopedClock({None: tick_clock.global_clock})
        )
        si = wait_nop.ins.sync_info
        if si is not None and si.on_wait and len(si.on_wait) > 1:
            waits = list(si.on_wait)
            si.on_wait = waits[:1]
            for w in waits[1:]:
                extra = nc.sync.nop(nofuse=True, hint="tile_tail_waits")
                xsi = extra.ins.sync_info
                if xsi is None:
                    extra.ins.sync_info = mybir.SyncInfo(on_wait=[w], on_update=[])
                else:
                    xsi.on_wait = [w]
        nc.sync.drain()
        nc.all_engine_barrier()
        assert self.sems is not None
        popped = nc._tile_sem_poison_stack.pop()
        assert popped is self._sem_poison
        nc.clear_and_free_semaphores(list(self.sems.allocated().values()))
        nc.all_engine_barrier()

    tile.TileContext._drain_and_barrier = _patched_drain_and_barrier
    _PATCHED[0] = True


def _split_multi_waits(nc):
    """Move extra sync-waits onto same-engine NoOps placed right before the
    carrying instruction (same per-engine program order => same gating)."""
    import concourse.mybir as mybir

    n = [0]
    for f in nc.m.functions:
        for bb in f.blocks:
            out = []
            changed = False
            for inst in bb.instructions:
                si = inst.sync_info
                if si is not None and si.on_wait and len(si.on_wait) > 1:
                    waits = list(si.on_wait)
                    si.on_wait = waits[:1]
                    for w in waits[1:]:
                        n[0] += 1
                        nop = mybir.InstNoOp(
                            name=f"waitsplit-{n[0]}",
                            engine=inst.engine,
                            ins=[],
                            outs=[],
                        )
                        nop.sync_info = mybir.SyncInfo(on_wait=[w], on_update=[])
                        out.append(nop)
                    changed = True
                out.append(inst)
            if changed:
                bb.instructions = out


# ----------------------------------------------------------------------------
# Device program
# ----------------------------------------------------------------------------


def build_program(order: int, rows: int, b2n: int, split_waits: bool = True):
    """Per-core program.

    Inputs (per core):
      a4t  (order, 128, rows) bf16 : steered query slabs, transposed, scaled -2
      d2t  (128, b2n)         bf16 : desc2 transposed
      b2r  (128, b2n)         f16  : desc2 squared norms, replicated over rows
      q2p  (128, order*NT)    f32  : query norms, tiled layout [i, s*NT+t]
    Output:
      cmin (NT*128, NG) f16 : per-(row, group) min of d^2 over all steers.
    """
    _apply_patches()
    import concourse.bass as bass
    import concourse.mybir as mybir
    import concourse.tile as tile

    f32 = mybir.dt.float32
    bf16 = mybir.dt.bfloat16
    f16 = mybir.dt.float16
    Alu = mybir.AluOpType
    Act = mybir.ActivationFunctionType

    NT = (rows + TILE_P - 1) // TILE_P
    NG = (b2n + GB - 1) // GB

    nc = bass.Bass("TRN2", target_bir_lowering=False, debug=False)
    a4t = nc.dram_tensor("a4t", [order, D, rows], bf16, kind="ExternalInput").ap()
    d2t = nc.dram_tensor("d2t", [D, b2n], bf16, kind="ExternalInput").ap()
    b2r = nc.dram_tensor("b2r", [TILE_P, b2n], f16, kind="ExternalInput").ap()
    q2p = nc.dram_tensor("q2p", [TILE_P, order * NT], f32, kind="ExternalInput").ap()
    cmin_d = nc.dram_tensor(
        "cmin", [NT * TILE_P, NG], f16, kind="ExternalOutput"
    ).ap()

    with tile.TileContext(nc) as tc:
        with (
            tc.tile_pool(name="const", bufs=1) as cpool,
            tc.tile_pool(name="work", bufs=3) as wpool,
            tc.tile_pool(name="psum", bufs=2, space="PSUM") as ppool,
        ):
            d2t_sb = cpool.tile([D, b2n], bf16, tag="d2t")
            nc.sync.dma_start(d2t_sb[:], d2t[:, :])
            b2r_sb = cpool.tile([TILE_P, b2n], f16, tag="b2r")
            nc.sync.dma_start(b2r_sb[:], b2r[:, :])
            q2_sb = cpool.tile([TILE_P, order * NT], f32, tag="q2p")
            nc.sync.dma_start(q2_sb[:], q2p[:, :])
            a_sb = []
            for s in range(order):
                t_ = cpool.tile([D, rows], bf16, tag=f"a{s}")
                nc.sync.dma_start(t_[:], a4t[s])
                a_sb.append(t_)

            for t in range(NT):
                P = min(TILE_P, rows - t * TILE_P)
                i0 = t * TILE_P
                strip = wpool.tile([TILE_P, NG], f16, tag="strip")

                def q2ap(s, P=P, t=t):
                    return q2_sb[:P, s * NT + t : s * NT + t + 1]

                def mm_group(p_, s, g0, w, P=P, i0=i0):
                    # fill (128, w) psum tile with -2*a_s . b[g0:g0+w]
                    for off in range(0, w, MMW):
                        ww = min(MMW, w - off)
                        nc.tensor.matmul(
                            p_[:P, off : off + ww],
                            a_sb[s][:, i0 : i0 + P],
                            d2t_sb[:, g0 + off : g0 + off + ww],
                            start=True,
                            stop=True,
                        )

                for g in range(NG):
                    g0 = g * GB
                    w = min(GB, b2n - g0)
                    if order == 4:
                        # half A (steers 0,1): ACT forms s1; DVE STT fuses
                        # s0's form with the pair min -> mA (f16)
                        pa = ppool.tile([TILE_P, GB], f32, tag="pgA")
                        mm_group(pa, 0, g0, w)
                        pb = ppool.tile([TILE_P, GB], f32, tag="pgB")
                        mm_group(pb, 1, g0, w)
                        t1 = wpool.tile([TILE_P, GB], f16, tag="t1")
                        nc.scalar.activation(
                            t1[:P, :w], pb[:P, :w], Act.Identity, bias=q2ap(1)
                        )
                        mA = wpool.tile([TILE_P, GB], f16, tag="mA")
                        nc.vector.scalar_tensor_tensor(
                            mA[:P, :w], pa[:P, :w], q2ap(0), t1[:P, :w],
                            Alu.add, Alu.min,
                        )
                        # half B (steers 2,3): ACT forms both; DVE f16 2x min
                        pa2 = ppool.tile([TILE_P, GB], f32, tag="pgA")
                        mm_group(pa2, 2, g0, w)
                        pb2 = ppool.tile([TILE_P, GB], f32, tag="pgB")
                        mm_group(pb2, 3, g0, w)
                        t2 = wpool.tile([TILE_P, GB], f16, tag="t2")
                        nc.scalar.activation(
                            t2[:P, :w], pa2[:P, :w], Act.Identity, bias=q2ap(2)
                        )
                        t3 = wpool.tile([TILE_P, GB], f16, tag="t3")
                        nc.scalar.activation(
                            t3[:P, :w], pb2[:P, :w], Act.Identity, bias=q2ap(3)
                        )
                        mB = wpool.tile([TILE_P, GB], f16, tag="mB")
                        nc.vector.tensor_tensor(
                            mB[:P, :w], t2[:P, :w], t3[:P, :w], Alu.min
                        )
                        u = wpool.tile([TILE_P, GB], f16, tag="u")
                        nc.vector.tensor_tensor(
                            u[:P, :w], mA[:P, :w], mB[:P, :w], Alu.min
                        )
                        d2c = wpool.tile([TILE_P, GB], f16, tag="d2c")
                        nc.gpsimd.tensor_tensor(
                            d2c[:P, :w], u[:P, :w], b2r_sb[:P, g0 : g0 + w], Alu.add
                        )
                        nc.vector.tensor_reduce(
                            strip[:P, g : g + 1], d2c[:P, :w],
                            mybir.AxisListType.X, Alu.min,
                        )
                    else:
                        # generic: single DVE min-chain over all steers (f32)
                        pa = ppool.tile([TILE_P, GB], f32, tag="pgA")
                        mm_group(pa, 0, g0, w)
                        uu = wpool.tile([TILE_P, GB], f32, tag="mA")
                        nc.vector.tensor_scalar(
                            uu[:P, :w], pa[:P, :w], q2ap(0), None, Alu.add
                        )
                        for s in range(1, order):
                            pb = ppool.tile([TILE_P, GB], f32, tag="pgB")
                            mm_group(pb, s, g0, w)
                            nc.vector.scalar_tensor_tensor(
                                uu[:P, :w], pb[:P, :w], q2ap(s), uu[:P, :w],
                                Alu.add, Alu.min,
                            )
                        d2c = wpool.tile([TILE_P, GB], f16, tag="d2c")
                        nc.vector.tensor_tensor(
                            d2c[:P, :w], uu[:P, :w], b2r_sb[:P, g0 : g0 + w], Alu.add
                        )
                        nc.vector.tensor_reduce(
                            strip[:P, g : g + 1], d2c[:P, :w],
                            mybir.AxisListType.X, Alu.min,
                        )

                nc.sync.dma_start(cmin_d[i0 : i0 + P, :], strip[:P, :])

    if split_waits:
        _split_multi_waits(nc)
    return nc


# ----------------------------------------------------------------------------
# Host side
# ----------------------------------------------------------------------------


def _host_prep(desc1, desc2, generator, order, n_cores):
    B1 = desc1.shape[0]
    B2 = desc2.shape[0]
    rows = B1 // n_cores
    NT = (rows + TILE_P - 1) // TILE_P

    a = desc1.astype(np.float32, copy=False)
    gT = generator.T.astype(np.float32, copy=False)
    steered = []
    for s in range(order):
        steered.append(a)
        if s + 1 < order:
            a = a @ gT
    q2 = np.stack([(x.astype(np.float32) ** 2).sum(1) for x in steered])  # (S, B1)
    b2 = (desc2.astype(np.float32) ** 2).sum(1)  # (B2,)

    import ml_dtypes

    d2t = np.ascontiguousarray(desc2.T.astype(ml_dtypes.bfloat16))
    b2r = np.ascontiguousarray(np.broadcast_to(b2, (TILE_P, B2)), dtype=np.float16)

    in_maps = []
    for k in range(n_cores):
        sl = slice(k * rows, (k + 1) * rows)
        a4t = np.stack(
            [np.ascontiguousarray((-2.0 * x[sl]).T.astype(ml_dtypes.bfloat16))
             for x in steered]
        )
        q2k = q2[:, sl]  # (S, rows)
        pad = NT * TILE_P - rows
        if pad:
            q2k = np.pad(q2k, ((0, 0), (0, pad)))
        q2p = np.ascontiguousarray(
            q2k.reshape(order, NT, TILE_P).transpose(2, 0, 1).reshape(TILE_P, order * NT)
        )
        in_maps.append({"a4t": a4t, "d2t": d2t, "b2r": b2r, "q2p": q2p})

    prep = dict(steered=steered, q2=q2, b2=b2, desc2=desc2, rows=rows, NT=NT,
                order=order, B1=B1, B2=B2, n_cores=n_cores)
    return in_maps, prep


def _postprocess(cmin_list, prep):
    """Exact re-rank: per row find groups whose approx min is within EPS of
    the row's global approx min, recompute those groups' d^2 exactly (fp32),
    take exact min + first argmin."""
    B1, B2 = prep["B1"], prep["B2"]
    rows, order = prep["rows"], prep["order"]
    steered, q2, b2, desc2 = prep["steered"], prep["q2"], prep["b2"], prep["desc2"]
    NG = (B2 + GB - 1) // GB

    strip = np.empty((B1, NG), np.float32)
    for k, cm in enumerate(cmin_list):
        strip[k * rows : (k + 1) * rows] = cm[:rows].astype(np.float32)

    gmin = strip.min(1)
    mask = strip <= (gmin + EPS)[:, None]

    best_v = np.full(B1, np.inf, np.float32)
    best_j = np.zeros(B1, np.int64)
    d2tT = desc2.T.astype(np.float32)  # (128, B2)
    for g in range(NG):
        ridx = np.nonzero(mask[:, g])[0]
        if ridx.size == 0:
            continue
        g0 = g * GB
        w = min(GB, B2 - g0)
        bT = d2tT[:, g0 : g0 + w]  # (128, w)
        blk = None
        for s in range(order):
            sv = steered[s][ridx]  # (n, 128)
            d2 = q2[s][ridx][:, None] + b2[None, g0 : g0 + w] - 2.0 * (sv @ bT)
            blk = d2 if blk is None else np.minimum(blk, d2)
        v = blk.min(1)
        j = blk.argmin(1) + g0
        upd = v < best_v[ridx]
        bi = ridx[upd]
        best_v[bi] = v[upd]
        best_j[bi] = j[upd]

    dists = np.sqrt(np.clip(best_v, 0.0, None)).astype(np.float32)
    return dists, best_j.astype(np.int32)


def run_cores(desc1, desc2, generator, order, n_cores=8, trace=False, trace_kwargs=None):
    from concourse.bass_utils import run_bass_kernel_spmd

    B2 = desc2.shape[0]
    in_maps, prep = _host_prep(desc1, desc2, generator, order, n_cores)

    key = (order, prep["rows"], B2)
    if key not in _CACHE:
        _CACHE[key] = build_program(order, prep["rows"], B2)
    nc = _CACHE[key]

    kw = {}
    if trace:
        kw = dict(trace=True, trace_kwargs=trace_kwargs or {})
    res = run_bass_kernel_spmd(nc, in_maps, core_ids=list(range(n_cores)), **kw)

    cmin_list = [res.results[k]["cmin"] for k in range(n_cores)]
    dists, idx, = _postprocess(cmin_list, prep)
    return dists, idx, res


def kernel(desc1, desc2, generator, steerer_order):
    order = int(steerer_order)
    desc1 = np.asarray(desc1, dtype=np.float32)
    desc2 = np.asarray(desc2, dtype=np.float32)
    generator = np.asarray(generator, dtype=np.float32)

    dists, idx, _ = run_cores(desc1, desc2, generator, order, n_cores=8)

    B1 = desc1.shape[0]
    idxs_in_1 = np.arange(B1, dtype=np.int32)
    matches = np.stack([idxs_in_1, idx], axis=1)
    return dists[:, None], matches
